# revision 63
# baseline (speedup 1.0000x reference)
"""LongT5 transient-global attention on 8 Trainium2 cores.

Sharding: core c = (batch b = c//4, sequence quarter qtr = c%4). Each core
computes the full output for its 1024 query tokens; K/V use a 1-block halo
(zero-padded at sequence edges); the 256 global summary tokens are computed
redundantly per core from the full batch hidden states.

Self-contained: hardcodes all shapes; host-side work is only data marshaling
and tiny bias-table precomputation (exp-free, bucketed rel-pos tables).

Device-side structure (per core):
  - hiddenT via XBAR DMA transposes straight from DRAM (no PE transposes)
  - Q/K/V/side projections in f16 with 4-chunk batched weight DMAs
  - global-block sums via tiny one-hot matmuls interleaved with QT
  - attention per 256-query strip: scores [key,1536] -> exp -> multiplicative
    exp-bias tables (DVE local / GPSIMD side) -> AV with ones-column
    denominator -> deferred reciprocal+normalize (software-pipelined so the
    DVE never head-of-line blocks on a PE wait)
  - attnT assembled feature-major via SBUF->SBUF XBAR DMA transposes
  - out-projection with th-outer ordering, bf16 output DMA
"""
import sys, math
sys.path.insert(0, "/opt/trn_rl_repo")
import numpy as np
import ml_dtypes

import concourse.bass as bass
import concourse.mybir as mybir
import concourse.tile as tile
from concourse import bacc
from concourse.bass_utils import run_bass_kernel_spmd

F32 = mybir.dt.float32
F16 = mybir.dt.float16
BF16 = mybir.dt.bfloat16

B, S, D = 2, 4096, 1024
H, DKV = 16, 64
L = 128                  # block len
G = 256                  # global tokens per batch (S/16)
GBLK = 16                # tokens per global block
NUM_BUCKETS, MAX_DIST = 32, 128
EPS = 1e-6
NEG = -30000.0           # fp16-safe mask value; exp() -> 0

TOK_Q = 1024             # query tokens per core
TOK_K = TOK_Q + 2 * L    # halo'd K/V tokens per core
NSTRIP = 4               # strips of 2 q-blocks
STRIP_Q = 256
GB_CORE = TOK_Q // GBLK  # 64 global-block ids per core
WT_W = 768               # local bias table width


class _GsumStepper:
    """Stepwise global block sums: .step() consumes one 128-token tile; tiles
    arrive four-at-a-time in batched DMAs. Pools are opened by the caller."""

    def __init__(self, nc, tc, hid_full, t_b16, ph, pg):
        self.nc, self.tc = nc, tc
        self.hid_full, self.t_b16, self.ph = hid_full, t_b16, ph
        self.giT_ps = pg.tile([L, 8 * G], F32, name="giT_ps")
        self.tt = 0
        self.htb = None

    def step(self):
        if self.tt >= S // L:
            return
        nc, tt = self.nc, self.tt
        if tt % 4 == 0:
            self.htb = self.ph.tile([L, 4 * D], F16, tag="hin", name=f"htb{tt // 4}")
            src = self.hid_full[tt * L:(tt + 4) * L, :].rearrange(
                "(i p) c -> p i c", i=4)
            nc.sync.dma_start(out=self.htb.rearrange("p (i c) -> p i c", i=4),
                              in_=src)
        base = (tt % 4) * D
        for dc in range(8):
            nc.tensor.matmul(
                self.giT_ps[:, dc * G + 8 * tt: dc * G + 8 * tt + 8],
                self.htb[:, base + dc * L: base + (dc + 1) * L], self.t_b16,
                start=True, stop=True)
        self.tt += 1

    def finish_sums(self):
        while self.tt < S // L:
            self.step()


def _phase_gsum_finish(nc, tc, giT_ps, t_lnw, gnT):
    """RMS norm of accumulated giT_ps -> gnT (feature-major, f32r)."""
    with tc.tile_pool(name="pssum", bufs=1, space="PSUM") as pssum, \
         tc.tile_pool(name="gtmp", bufs=1) as pgt:
        giT = pgt.tile([L, 8 * G], F32, tag="giT")
        nc.vector.tensor_copy(out=giT, in_=giT_ps)
        sq = pgt.tile([L, 8 * G], F32, tag="sq")
        nc.vector.tensor_mul(out=sq, in0=giT, in1=giT)
        ones1 = pgt.tile([L, 1], F32, tag="ones1")
        nc.vector.memset(ones1, 1.0)
        ps_small = pssum.tile([L, 512], F32)
        ssum = ps_small[0:1, 0:G]
        for dc in range(8):
            nc.tensor.matmul(ssum, ones1, sq[:, dc * G:(dc + 1) * G],
                             start=(dc == 0), stop=(dc == 7))
        eps_t = pgt.tile([1, 1], F32, tag="eps")
        nc.vector.memset(eps_t, EPS)
        sd = pgt.tile([1, G], F32, tag="sd")
        nc.scalar.activation(out=sd, in_=ssum,
                             func=mybir.ActivationFunctionType.Sqrt,
                             bias=eps_t, scale=1.0 / D)
        rstd = pgt.tile([1, G], F32, tag="rstd")
        nc.vector.reciprocal(out=rstd, in_=sd)
        ones_row = pgt.tile([1, L], F32, tag="onesrow")
        nc.vector.memset(ones_row, 1.0)
        rstd_w = ps_small[:, 256:256 + G]
        nc.tensor.matmul(rstd_w, ones_row, rstd, start=True, stop=True)
        for dc in range(8):
            nc.vector.tensor_mul(out=giT[:, dc * G:(dc + 1) * G],
                                 in0=giT[:, dc * G:(dc + 1) * G], in1=rstd_w)
            nc.vector.tensor_scalar_mul(giT[:, dc * G:(dc + 1) * G],
                                        in0=giT[:, dc * G:(dc + 1) * G],
                                        scalar1=t_lnw[:, dc:dc + 1])
        nc.vector.tensor_copy(out=gnT, in_=giT)
        # Hoist the Exp activation-table load into the proj phase (ACT is
        # idle here; saves the table-load stall at the first attention exp).
        warm = pgt.tile([1, 1], F32, tag="warm")
        nc.scalar.activation(out=warm, in_=eps_t,
                             func=mybir.ActivationFunctionType.Exp)


def _phase_proj(nc, tc, hid_k, wq, wk, wv, gnT, QT, KT, sideKT,
                V_aug, sideV_aug, hid_full=None, t_b16=None, t_lnw=None,
                tab_dmas=None):
    """hiddenT DMA-transpose + Q/K/V/side projections, with gsum interleaved."""
    with tc.tile_pool(name="hT", bufs=1) as phT, \
         tc.tile_pool(name="pw5", bufs=8) as pw5, \
         tc.tile_pool(name="hin", bufs=3) as ph_gs, \
         tc.tile_pool(name="pg", bufs=1, space="PSUM") as pg_gs:
        gs = _GsumStepper(nc, tc, hid_full, t_b16, ph_gs, pg_gs)
        hiddenT = phT.tile([L, 8 * TOK_K], F16)

        def load_w(wdram, fg, tag, j):
            # One DMA for 4 d-chunks of a 512-wide feature group:
            # tile[p, i*512 + c] = wdram[(j*4+i)*128 + p, fg*512 + c]
            t = pw5.tile([L, 4 * 512], F16, tag=tag, name=f"{tag}_{fg}_{j}")
            src = wdram[j * 512:(j + 1) * 512, fg * 512:(fg + 1) * 512].rearrange(
                "(i p) c -> p i c", i=4)
            nc.sync.dma_start(out=t.rearrange("p (i c) -> p i c", i=4), in_=src)
            return t

        def wsl(wt, dc, c0, cw):
            # 128-col slice [c0, c0+cw) of weight chunk dc from paired tiles
            return wt[dc // 4][:, (dc % 4) * 512 + c0:(dc % 4) * 512 + c0 + cw]

        # gsum tile 0 DMA rides first so PE has work immediately.
        gs.step()
        # Prefetch the first Q weight group on SP before anything else queues.
        wq_t = {0: [load_w(wq, 0, "wvo", j) for j in range(2)]}
        # XBAR DMA transpose: hiddenT[p, dc*TOK_K + t] = hid_k[t, dc*128+p],
        # in 5 token chunks of 256, ordered by first use (QT th0 reads
        # tokens 128..640 = chunks 1,2; chunk 0 is only needed by KT).
        for ch in range(5):
            t0 = ch * 256
            dst = bass.AP(tensor=hiddenT.tensor,
                          offset=hiddenT.offset + t0,
                          ap=[[hiddenT.ap[0][0], L], [TOK_K, 8], [1, 256]])
            nc.sync.dma_start_transpose(dst, hid_k[t0:t0 + 256, :])
        wq_t[1] = [load_w(wq, 1, "wvo", j) for j in range(2)]
        # K weights prefetch early (needed right after QT).
        wk_t = {fg: [load_w(wk, fg, "wvo", j) for j in range(2)] for fg in range(2)}
        # Bias tables queue last among the early DMAs (needed only at attn).
        if tab_dmas:
            for dst_t, src_t in tab_dmas:
                nc.sync.dma_start(out=dst_t, in_=src_t[:])
        # PE warms up on gsum tiles while the transposes land.
        for _ in range(7):
            gs.step()

        ppj_ctx = tc.tile_pool(name="ppj", bufs=3, space="PSUM")
        ppj = ppj_ctx.__enter__()
        # ---- QT (512-wide weight tiles, sliced per fc) ----
        for fg in range(2):
            for fl in range(4):
                fc = fg * 4 + fl
                for th in range(2):
                    pq = ppj.tile([L, 512], F32, tag="ppj", name=f"pq{fc}_{th}")
                    for dc in range(8):
                        nc.tensor.matmul(
                            pq, wsl(wq_t[fg], dc, fl * L, L),
                            hiddenT[:, dc * TOK_K + L + th * 512: dc * TOK_K + L + (th + 1) * 512],
                            start=(dc == 0), stop=(dc == 7))
                    nc.vector.tensor_copy(out=QT[:, fc * TOK_Q + th * 512: fc * TOK_Q + (th + 1) * 512], in_=pq)
                    gs.step()
        # ---- KT local tokens (both weight groups stay resident in pw5);
        # gsum RMS-norm (DVE-heavy) is emitted between the two fg groups so
        # it overlaps KT matmuls on PE; sideKT afterwards needs gnT. ----
        for fg in range(2):
            for fl in range(4):
                fc = fg * 4 + fl
                for th in range(3):
                    w_ = 512 if th < 2 else 256
                    pk = ppj.tile([L, 512], F32, tag="ppj", name=f"pk{fc}_{th}")
                    for dc in range(8):
                        nc.tensor.matmul(
                            pk[:, :w_], wsl(wk_t[fg], dc, fl * L, L),
                            hiddenT[:, dc * TOK_K + th * 512: dc * TOK_K + th * 512 + w_],
                            start=(dc == 0), stop=(dc == 7))
                    nc.vector.tensor_copy(out=KT[:, fc * TOK_K + th * 512: fc * TOK_K + th * 512 + w_],
                                   in_=pk[:, :w_])
                    gs.step()
            if fg == 0:
                gs.finish_sums()
                _phase_gsum_finish(nc, tc, gs.giT_ps, t_lnw, gnT)
        for fg in range(2):
            for fl in range(4):
                fc = fg * 4 + fl
                psk = ppj.tile([L, 512], F32, tag="ppj", name=f"psk{fc}")
                for dc in range(8):
                    nc.tensor.matmul(psk[:, :G], wsl(wk_t[fg], dc, fl * L, L),
                                     gnT[:, dc * G:(dc + 1) * G],
                                     start=(dc == 0), stop=(dc == 7))
                nc.vector.tensor_copy(out=sideKT[:, fc * G:(fc + 1) * G], in_=psk[:, :G])
        # ---- V + sideV (token-major bf16 with ones column) ----
        for fh in range(2):
            wv_t = [load_w(wv, fh, "wvo", j) for j in range(2)]
            for tt in range(10):
                pv = ppj.tile([L, 512], F32, tag="ppj", name=f"pv{fh}_{tt}")
                for dc in range(8):
                    nc.tensor.matmul(
                        pv, hiddenT[:, dc * TOK_K + tt * L: dc * TOK_K + (tt + 1) * L],
                        wsl(wv_t, dc, 0, 512), start=(dc == 0), stop=(dc == 7))
                dst = bass.AP(tensor=V_aug[tt].tensor,
                              offset=V_aug[tt].offset + fh * 8 * (DKV + 1),
                              ap=[[V_aug[tt].ap[0][0], L], [DKV + 1, 8], [1, DKV]])
                nc.vector.tensor_copy(out=dst, in_=pv.rearrange("p (h d) -> p h d", h=8))
            for gt in range(2):
                pv = ppj.tile([L, 512], F32, tag="ppj", name=f"psv{fh}_{gt}")
                for dc in range(8):
                    nc.tensor.matmul(
                        pv, gnT[:, dc * G + gt * L: dc * G + (gt + 1) * L],
                        wsl(wv_t, dc, 0, 512), start=(dc == 0), stop=(dc == 7))
                dst = bass.AP(tensor=sideV_aug[gt].tensor,
                              offset=sideV_aug[gt].offset + fh * 8 * (DKV + 1),
                              ap=[[sideV_aug[gt].ap[0][0], L], [DKV + 1, 8], [1, DKV]])
                nc.vector.tensor_copy(out=dst, in_=pv.rearrange("p (h d) -> p h d", h=8))
        ppj_ctx.__exit__(None, None, None)


def _phase_attn(nc, tc, t_wtab, t_sideb, QT, KT, sideKT, V_aug, sideV_aug,
                attnT, h_used, strips, pet, pat, psc):
    """Per-strip (256 queries) attention: 4 local + 2 side key chunks.

    Scores are [key(128-part), 1536] per (strip, head); softmax denominator
    comes from the ones column of V_aug; attnT is assembled feature-major
    via XBAR DMA transposes."""
    with tc.tile_pool(name="pst", bufs=2, space="PSUM") as pst_pool, \
         tc.tile_pool(name="ppv", bufs=2, space="PSUM") as ppv_pool:
        wt_pstride = t_wtab.ap[0][0]
        sb_pstride = t_sideb.ap[0][0]

        def normalize(pend):
            # deferred by one iteration so DVE's recip never head-of-line
            # blocks the next head's bias multiplies behind a PE wait
            pv_ps, sb, h = pend
            rec = psc.tile([L, 2], F32, tag="rec", name=f"rec_n{h}")
            den = bass.AP(tensor=pv_ps.tensor, offset=pv_ps.offset + DKV,
                          ap=[[pv_ps.ap[0][0], L], [DKV + 1, 2]])
            nc.vector.reciprocal(out=rec, in_=den)
            for qh in range(2):
                nc.vector.tensor_scalar_mul(
                    sb[qh][:, h * DKV:(h + 1) * DKV],
                    in0=pv_ps[:, qh * (DKV + 1): qh * (DKV + 1) + DKV],
                    scalar1=rec[:, qh:qh + 1])

        pending = None
        for strip in strips:
            attn_sb = [pat.tile([L, D], F16, tag=f"attn{i}", name=f"attn_{strip}_{i}")
                       for i in range(2)]
            if h_used < H:
                for i in range(2):
                    nc.vector.memset(attn_sb[i], 0.0)
            for h in range(h_used):
                fc, p0 = h // 2, (h % 2) * DKV
                st = pst_pool.tile([L, 1536], F32, tag="st", name=f"st{strip}_{h}")
                qt_ap = QT[p0:p0 + DKV,
                           fc * TOK_Q + strip * STRIP_Q: fc * TOK_Q + (strip + 1) * STRIP_Q]
                for c in range(4):
                    kstart = strip * STRIP_Q + c * L
                    nc.tensor.matmul(
                        st[:, c * STRIP_Q:(c + 1) * STRIP_Q],
                        KT[p0:p0 + DKV, fc * TOK_K + kstart: fc * TOK_K + kstart + L],
                        qt_ap, start=True, stop=True)
                for c in range(2):
                    nc.tensor.matmul(
                        st[:, (4 + c) * STRIP_Q:(5 + c) * STRIP_Q],
                        sideKT[p0:p0 + DKV, fc * G + c * L: fc * G + (c + 1) * L],
                        qt_ap, start=True, stop=True)
                et = pet.tile([L, 1536], BF16, tag="et", name=f"et{strip}_{h}")
                nc.scalar.activation(out=et, in_=st,
                                     func=mybir.ActivationFunctionType.Exp)
                # multiplicative biases (tables hold exp(bias); 0 = masked)
                loc = bass.AP(tensor=t_wtab.tensor,
                              offset=t_wtab.offset + h * WT_W + 255,
                              ap=[[wt_pstride, L], [L, 4], [-1, STRIP_Q]])
                nc.vector.tensor_mul(
                    out=et[:, 0:1024].rearrange("p (c q) -> p c q", c=4),
                    in0=et[:, 0:1024].rearrange("p (c q) -> p c q", c=4),
                    in1=loc)
                sid = bass.AP(tensor=t_sideb.tensor,
                              offset=t_sideb.offset + h * GB_CORE + strip * 16,
                              ap=[[sb_pstride, L], [H * GB_CORE, 2], [1, 16], [0, 16]])
                nc.gpsimd.tensor_mul(
                    out=et[:, 1024:1536].rearrange("p (c b r) -> p c b r", c=2, b=16),
                    in0=et[:, 1024:1536].rearrange("p (c b r) -> p c b r", c=2, b=16),
                    in1=sid)
                pv_ps = ppv_pool.tile([L, 2 * (DKV + 1)], F32, tag="pv",
                                      name=f"pv{strip}_{h}")
                for qh in range(2):
                    for c in range(6):
                        if c < 4:
                            rhs = V_aug[strip * 2 + c][:, h * (DKV + 1): (h + 1) * (DKV + 1)]
                        else:
                            rhs = sideV_aug[c - 4][:, h * (DKV + 1): (h + 1) * (DKV + 1)]
                        nc.tensor.matmul(
                            pv_ps[:, qh * (DKV + 1):(qh + 1) * (DKV + 1)],
                            et[:, c * STRIP_Q + qh * L: c * STRIP_Q + qh * L + L],
                            rhs, start=(c == 0), stop=(c == 5))
                if pending is not None:
                    normalize(pending)
                pending = (pv_ps, attn_sb, h)
            # strip boundary: attn_sb must be complete before its transpose
            if pending is not None:
                normalize(pending)
                pending = None
            for qh in range(2):
                tt = strip * 2 + qh
                dstT = bass.AP(tensor=attnT.tensor,
                               offset=attnT.offset + tt * L,
                               ap=[[attnT.ap[0][0], L], [TOK_Q, 8], [1, L]])
                nc.sync.dma_start_transpose(dstT, attn_sb[qh][:, :])
def _phase_outproj(nc, tc, wo_t, attnT, outT):
    with tc.tile_pool(name="ppo", bufs=4, space="PSUM") as ppo, \
         tc.tile_pool(name="pout", bufs=4) as pout:
        for th in range(2):
            for ng in range(2):
                for nl in range(4):
                    nc_out = ng * 4 + nl
                    po = ppo.tile([L, 512], F32, tag="ppo", name=f"po{nc_out}_{th}")
                    for ic in range(8):
                        wsl_o = wo_t[ng][ic // 4][:, (ic % 4) * 512 + nl * L:
                                                  (ic % 4) * 512 + (nl + 1) * L]
                        nc.tensor.matmul(
                            po, wsl_o,
                            attnT[:, ic * TOK_Q + th * 512: ic * TOK_Q + (th + 1) * 512],
                            start=(ic == 0), stop=(ic == 7))
                    ot = pout.tile([L, 512], BF16, tag="ot", name=f"ot{nc_out}_{th}")
                    nc.vector.tensor_copy(out=ot, in_=po)
                    nc.sync.dma_start(
                        out=outT[nc_out * L:(nc_out + 1) * L, th * 512:(th + 1) * 512],
                        in_=ot)


def _build_nc(h_used=H, nstrip=NSTRIP, zero_attnT=False):
    nc = bacc.Bacc(None, target_bir_lowering=False, debug=False)

    hid_k = nc.declare_dram_parameter("hid_k", [TOK_K, D], F16, isOutput=False)
    hid_full = nc.declare_dram_parameter("hid_full", [S, D], F16, isOutput=False)
    wq = nc.declare_dram_parameter("wq", [D, D], F16, isOutput=False)
    wk = nc.declare_dram_parameter("wk", [D, D], F16, isOutput=False)
    wv = nc.declare_dram_parameter("wv", [D, D], F16, isOutput=False)
    wo = nc.declare_dram_parameter("wo", [D, D], F16, isOutput=False)
    b16 = nc.declare_dram_parameter("b16", [L, 8], F16, isOutput=False)
    wtab = nc.declare_dram_parameter("wtab", [L, H * WT_W], F16, isOutput=False)
    sideb = nc.declare_dram_parameter("sideb", [L, 2 * H * GB_CORE], F16, isOutput=False)
    lnw = nc.declare_dram_parameter("lnw", [L, 8], F32, isOutput=False)
    outT = nc.declare_dram_parameter("outT", [D, TOK_Q], BF16, isOutput=True)

    with tile.TileContext(nc) as tc:
        with tc.tile_pool(name="persist", bufs=1) as pp_sb, \
             tc.tile_pool(name="acts", bufs=1) as pa:
            t_b16 = pp_sb.tile([L, 8], F16)
            t_lnw = pp_sb.tile([L, 8], F32)
            t_wtab = pp_sb.tile([L, H * WT_W], F16)
            t_sideb = pp_sb.tile([L, 2 * H * GB_CORE], F16)
            nc.sync.dma_start(out=t_b16, in_=b16[:])
            nc.sync.dma_start(out=t_lnw, in_=lnw[:])

            QT = pa.tile([L, 8 * TOK_Q], F16)      # (feat-part, fc x tok)
            KT = pa.tile([L, 8 * TOK_K], F16)
            sideKT = pa.tile([L, 8 * G], F16)
            V_aug = [pa.tile([L, H * (DKV + 1)], BF16, tag=f"vaug{t}", name=f"vaug{t}")
                     for t in range(10)]
            sideV_aug = [pa.tile([L, H * (DKV + 1)], BF16, tag=f"svaug{t}", name=f"svaug{t}")
                         for t in range(2)]
            for t in range(10):
                ones_ap = bass.AP(tensor=V_aug[t].tensor,
                                  offset=V_aug[t].offset + DKV,
                                  ap=[[V_aug[t].ap[0][0], L], [DKV + 1, H]])
                nc.gpsimd.memset(ones_ap, 1.0)
            for t in range(2):
                ones_ap = bass.AP(tensor=sideV_aug[t].tensor,
                                  offset=sideV_aug[t].offset + DKV,
                                  ap=[[sideV_aug[t].ap[0][0], L], [DKV + 1, H]])
                nc.gpsimd.memset(ones_ap, 1.0)

            with tc.tile_pool(name="pmid", bufs=1) as pmid:
                gnT = pmid.tile([L, 8 * G], F16)  # (D-part, dc x g)
                _phase_proj(nc, tc, hid_k, wq, wk, wv, gnT,
                            QT, KT, sideKT, V_aug, sideV_aug,
                            hid_full=hid_full, t_b16=t_b16, t_lnw=t_lnw,
                            tab_dmas=[(t_wtab, wtab), (t_sideb, sideb)])

            with tc.tile_pool(name="pattnT", bufs=1) as pan, \
                 tc.tile_pool(name="pw2", bufs=4) as pw2:
                attnT = pan.tile([L, 8 * TOK_Q], F16)
                if zero_attnT:
                    nc.vector.memset(attnT, 0.0)
                wo_t = {}
                for ng in range(2):
                    wo_t[ng] = []
                    for j in range(2):
                        t = pw2.tile([L, 4 * 512], F16, tag="wo", name=f"wo_{ng}_{j}")
                        src = wo[j * 512:(j + 1) * 512,
                                 ng * 512:(ng + 1) * 512].rearrange(
                            "(i p) c -> p i c", i=4)
                        nc.sync.dma_start(out=t.rearrange("p (i c) -> p i c", i=4),
                                          in_=src)
                        wo_t[ng].append(t)
                with tc.tile_pool(name="pet", bufs=4) as pet, \
                     tc.tile_pool(name="pat", bufs=4) as pat, \
                     tc.tile_pool(name="psc", bufs=6) as psc:
                    _phase_attn(nc, tc, t_wtab, t_sideb, QT, KT, sideKT,
                                V_aug, sideV_aug, attnT, h_used,
                                list(range(NSTRIP)), pet, pat, psc)
                _phase_outproj(nc, tc, wo_t, attnT, outT)

    nc.finalize()
    return nc


# ---------------- host-side table construction ----------------

def _rel_bucket_np(rp):
    """Bit-faithful port of reference _rel_bucket via jax f32 on CPU.

    Must run on CPU: the axon/neuron backend's log() uses activation-table
    approximations that flip int32-truncated bucket boundaries."""
    import jax
    import jax.numpy as jnp
    with jax.default_device(jax.devices("cpu")[0]):
        rp = jnp.asarray(rp)
        nb = NUM_BUCKETS // 2
        buckets = jnp.where(rp > 0, nb, 0).astype(jnp.int32)
        rpa = jnp.abs(rp)
        max_exact = nb // 2
        is_small = rpa < max_exact
        rp_f = jnp.maximum(rpa, 1).astype(jnp.float32)
        rp_large = max_exact + (jnp.log(rp_f / max_exact) / math.log(MAX_DIST / max_exact)
                                * (nb - max_exact)).astype(jnp.int32)
        rp_large = jnp.minimum(rp_large, nb - 1)
        out = buckets + jnp.where(is_small, rpa.astype(jnp.int32), rp_large)
        return np.asarray(out)


def _make_tables(rel_bias, global_rel_bias, qtr):
    # local: W_h[i] for delta = i-383 in [-383, 384]
    delta = np.arange(WT_W) - 383
    buck = _rel_bucket_np(delta)
    wvals = np.exp(rel_bias[buck, :].astype(np.float64)).astype(np.float32)  # (768, H)
    wvals[np.abs(delta) >= L, :] = 0.0
    wtab = np.empty((L, H * WT_W), np.float16)
    idx = np.minimum(np.arange(WT_W)[None, :] + np.arange(L)[:, None], WT_W - 1)
    for h in range(H):
        wtab[:, h * WT_W:(h + 1) * WT_W] = wvals[idx, h].astype(np.float16)
    # side: sideb[p, gc*H*GB + h*GB + gb] = grel[bucket(g - (qtr*64+gb)), h]
    g = np.arange(G)
    gb_abs = qtr * GB_CORE + np.arange(GB_CORE)
    srel = g[:, None] - gb_abs[None, :]           # (256, 64)
    sbuck = _rel_bucket_np(srel)
    svals = np.exp(global_rel_bias[sbuck, :].astype(np.float64)).astype(np.float32)  # (256, 64, H)
    sideb = np.empty((L, 2 * H * GB_CORE), np.float16)
    for gc in range(2):
        for h in range(H):
            sideb[:, gc * H * GB_CORE + h * GB_CORE: gc * H * GB_CORE + (h + 1) * GB_CORE] = \
                svals[gc * L:(gc + 1) * L, :, h].astype(np.float16)
    return wtab, sideb


_NC_CACHE = {}


def kernel(hidden_states, mask, Wq, Wk, Wv, Wo, rel_bias, global_rel_bias, ln_weight):
    hidden_states = np.asarray(hidden_states, np.float32)
    Wq, Wk, Wv, Wo = (np.asarray(w, np.float32) for w in (Wq, Wk, Wv, Wo))
    rel_bias = np.asarray(rel_bias, np.float32)
    global_rel_bias = np.asarray(global_rel_bias, np.float32)
    ln_weight = np.asarray(ln_weight, np.float32)

    if "nc" not in _NC_CACHE:
        _NC_CACHE["nc"] = _build_nc()
    nc = _NC_CACHE["nc"]

    b16 = np.zeros((L, 8), np.float16)
    for t in range(L):
        b16[t, t // GBLK] = 1.0
    lnw = ln_weight.reshape(8, L).T.copy()        # lnw[p, dc] = ln_weight[dc*128+p]

    in_maps = []
    for c in range(8):
        b, qtr = c // 4, c % 4
        lo = qtr * TOK_Q - L
        hk = np.zeros((TOK_K, D), np.float16)
        s0, s1 = max(lo, 0), min(lo + TOK_K, S)
        hk[s0 - lo: s1 - lo] = hidden_states[b, s0:s1]
        wtab, sideb = _make_tables(rel_bias, global_rel_bias, qtr)
        in_maps.append({
            "hid_k": hk, "hid_full": hidden_states[b].astype(np.float16),
            "wq": Wq.astype(np.float16), "wk": Wk.astype(np.float16),
            "wv": Wv.astype(np.float16), "wo": Wo.astype(np.float16),
            "b16": b16, "wtab": wtab, "sideb": sideb, "lnw": lnw,
        })

    res = run_bass_kernel_spmd(nc, in_maps, core_ids=list(range(8)))
    out = np.empty((B, S, D), np.float32)
    for c in range(8):
        b, qtr = c // 4, c % 4
        out[b, qtr * TOK_Q:(qtr + 1) * TOK_Q, :] = \
            res.results[c]["outT"].astype(np.float32).T
    return out

